# revision 11
# baseline (speedup 1.0000x reference)
"""BertAttention (B=2,S=2048,D=1024,H=16) on 8 trn2 NeuronCores — v2 (fp8).

Sharding: data-parallel over B (2 groups of 4 cores); each group's 4 cores
split the 2048 query rows (512 each). Every core computes K^T and V for its
batch in full (redundant within the group), its own 512-row Q slice,
attention over all 16 heads for its rows, output projection, residual and
LayerNorm. No collectives; each core emits a disjoint [512, 1024] output
slice.

v2 changes vs the bf16 baseline:
  - all matmuls are fp8 (e4m3) with DoubleRow perf mode (2 fp8 MACs per PE
    cell per cycle). Weights are pre-scaled by 32 on the host to lift them
    out of the fp8 denormal range; the scale is compensated downstream.
  - Q^T/K^T live in a head-grouped layout ([32-partition band per h%4,
    hd split in two free slots]) so the score matmuls contract hd=64 as
    DoubleRow [32,2,*] operands. A host-side column permutation of Wq/Wk
    makes the projection PSUM slabs land partition-aligned.
  - per-head PV accumulates over all 16 key tiles in a single PSUM bank
    (no SBUF partial accumulation); V rows carry a denominator column
    (em/8) so PV row 64 is the softmax denominator.
  - QKV biases enter via rank-1 matmuls (bias_row^T @ ones) on the PE,
    so PSUM->SBUF evacuation is a pure convert-copy that can run on
    either ACT or DVE.
  - exp is split across the Scalar engine (native Exp -> fp8) and the
    Vector engine (Schraudolph bit-trick: int8(a*x+b) bitcast to fp8e4,
    exact softmax invariance under the uniform part of the bias, ~3%
    elementwise; attention output is ~1% of the residual so this is far
    inside the 2e-2 gate). Assignment is greedy by modeled op cost.
  - softmax reciprocal is batched (8 heads per DVE reciprocal) and
    broadcast across partitions via a DRAM-bounce DMA, then applied by
    GpSimd (SBUF-only engine) from a DMA-parked copy of the PV PSUM.
"""

import numpy as np

B, S, D, H = 2, 2048, 1024, 16
HD = D // H  # 64
P = 128
NCORES = 8
SQ = S // 4  # 512 query rows per core
DT = D // P  # 8 feature tiles
KS = S // P  # 16 key tiles (128 keys each)
NG = H // 4  # 4 head groups (4 heads stacked per partition dim)
EPS = 1e-12

WSCALE = 32.0  # weight pre-scale (fp8 denormal avoidance)
F8MAX = 240.0  # TRN fp8e4 max normal (above: Inf!)
ALPHA = 1.0 / (WSCALE * WSCALE * 8.0)  # exp scale on raw score psum (2^-13)
EXP_A = 8.0 * 1.4426950408889634 * ALPHA  # DVE bit-trick multiplier
EXP_B = 56.344  # DVE bit-trick bias (8*(7-0.043... minimax))
CTX_S = 256.0  # ctxn = CTX_S * ctx (fp8 range placement)
DCOL = WSCALE / CTX_S  # 1/8: V denominator-column scale
OSCALE = 1.0 / (WSCALE * CTX_S)  # 1/8192: O-proj psum descale

_CACHE = {}


def _ensure_paths():
    try:
        import concourse  # noqa: F401
    except ImportError:
        import sys

        for p in ("/opt/trn_rl_repo", "/root/.axon_site/_ro/trn_rl_repo"):
            if p not in sys.path:
                sys.path.append(p)
        import concourse  # noqa: F401


def _perm():
    """Column permutation for Wq/Wk: psum slab (dt) partition p holds
    feature d = (4*(dt//2) + p//32)*64 + (dt%2)*32 + p%32."""
    idx = np.empty(D, dtype=np.int64)
    for dtc in range(DT):
        g, half = dtc // 2, dtc % 2
        for p in range(P):
            h = 4 * g + p // 32
            hd = half * 32 + (p % 32)
            idx[dtc * P + p] = h * HD + hd
    return idx


def build_nc():
    _ensure_paths()
    import concourse.tile as tile
    from concourse import bacc, mybir

    f32 = mybir.dt.float32
    bf16 = mybir.dt.bfloat16
    f8 = mybir.dt.float8e4
    i8 = mybir.dt.int8
    DR = mybir.MatmulPerfMode.DoubleRow
    EXP = mybir.ActivationFunctionType.Exp
    COPY = mybir.ActivationFunctionType.Copy
    MULT = mybir.AluOpType.mult
    ADD = mybir.AluOpType.add

    nc = bacc.Bacc()

    # ---- I/O ----
    xT = nc.declare_dram_parameter("xT", [D, S], f8, isOutput=False)
    xq = nc.declare_dram_parameter("xq", [SQ, D], f32, isOutput=False)
    Wq = nc.declare_dram_parameter("Wq", [D, D], f8, isOutput=False)
    Wk = nc.declare_dram_parameter("Wk", [D, D], f8, isOutput=False)
    Wv = nc.declare_dram_parameter("Wv", [D, D], f8, isOutput=False)
    Wo = nc.declare_dram_parameter("Wo", [D, D], f8, isOutput=False)
    bq_r = nc.declare_dram_parameter("bq_r", [1, D], bf16, isOutput=False)
    bk_r = nc.declare_dram_parameter("bk_r", [1, D], bf16, isOutput=False)
    bv_r = nc.declare_dram_parameter("bv_r", [1, D], bf16, isOutput=False)
    em_t = nc.declare_dram_parameter("em_t", [P, KS], f32, isOutput=False)
    em8_t = nc.declare_dram_parameter("em8_t", [P, KS], f32, isOutput=False)
    gamma_bc = nc.declare_dram_parameter("gamma_bc", [P, D], f32, isOutput=False)
    beta_bc = nc.declare_dram_parameter("beta_bc", [P, D], f32, isOutput=False)
    out = nc.declare_dram_parameter("out", [SQ, D], f32, isOutput=True)

    sums_dram = nc.dram_tensor("sums_bounce", [H, SQ], f32)

    xT_r = xT.rearrange("(t p) s -> p t s", p=P)
    W_r = {
        "q": Wq.rearrange("(t p) d -> p t d", p=P),
        "k": Wk.rearrange("(t p) d -> p t d", p=P),
        "v": Wv.rearrange("(t p) d -> p t d", p=P),
        "o": Wo.rearrange("(t p) d -> p t d", p=P),
    }
    xq_r = xq.rearrange("(t p) d -> p t d", p=P)  # [128, 4, 1024]
    out_r = out.rearrange("(t p) d -> t p d", p=P)  # [4, 128, 1024]

    def mm(ps, lhsT, rhs, start, stop, dr=True, tile_position=None):
        nc.tensor.matmul(
            ps, lhsT, rhs, start=start, stop=stop,
            perf_mode=DR if dr else None,
            tile_position=tile_position,
        )

    # greedy ACT/DVE load balancing (modeled op costs in µs)
    eng_t = [0.0, 8.0]  # [ACT, DVE]; DVE pre-loaded with its fixed late work

    def pick(cost_act, cost_dve):
        if eng_t[0] + cost_act <= eng_t[1] + cost_dve:
            eng_t[0] += cost_act
            return 0
        eng_t[1] += cost_dve
        return 1

    with tile.TileContext(nc) as tc:
        with (
            tc.tile_pool(name="consts", bufs=1) as consts,
            tc.tile_pool(name="pers", bufs=1) as pers,
            tc.tile_pool(name="exp", bufs=4) as ex_pool,
            tc.tile_pool(name="bcast", bufs=2) as bc_pool,
        ):
            # ---- constants / inputs ----
            xt_sb = pers.tile([P, DT, S], f8)
            nc.sync.dma_start(xt_sb[:], xT_r[:])
            wq_sb = pers.tile([P, DT, D], f8)
            nc.sync.dma_start(wq_sb[:], W_r["q"][:])
            wk_sb = pers.tile([P, DT, D], f8)
            nc.sync.dma_start(wk_sb[:], W_r["k"][:])
            wv_sb = pers.tile([P, DT, D], f8)
            nc.sync.dma_start(wv_sb[:], W_r["v"][:])

            ones_sb = consts.tile([1, D], bf16)
            nc.gpsimd.memset(ones_sb[:], 1.0)
            bq_sb = consts.tile([1, D], bf16)
            nc.sync.dma_start(bq_sb[:], bq_r[:])
            bk_sb = consts.tile([1, D], bf16)
            nc.sync.dma_start(bk_sb[:], bk_r[:])
            bv_sb = consts.tile([1, D], bf16)
            nc.sync.dma_start(bv_sb[:], bv_r[:])
            em_sb = consts.tile([P, KS], f32)
            nc.sync.dma_start(em_sb[:], em_t[:])
            em8_sb = consts.tile([P, KS], f32)
            nc.sync.dma_start(em8_sb[:], em8_t[:])

            # persistent activation tiles
            qt = pers.tile([P, NG, 2, SQ], f8)  # Q^T head-grouped
            kt = pers.tile([P, NG, 2, S], f8)  # K^T head-grouped
            v_sb = pers.tile([P, KS // 2, 2, H, HD + 1], f8)
            ctxn = pers.tile([P, DT, SQ], f8)  # 256*ctx^T (normalized)
            apark = pers.tile([HD + 1, H, SQ], f32)  # parked PV psums
            # softmax denominators, one tile per 8-head batch (engine ops
            # must start at partition 0)
            sums_sb = [
                pers.tile([8, SQ], f32, name=f"sums{i}") for i in range(2)
            ]
            xq_sb = pers.tile([P, 4, D], f32)
            nc.sync.dma_start(xq_sb[:], xq_r[:])
            wo_sb = pers.tile([P, DT, D], f8)
            nc.sync.dma_start(wo_sb[:], W_r["o"][:])

            def evac(dst, src, scale=None):
                """PSUM->SBUF convert-copy on ACT or DVE (greedy)."""
                fd = src.free_size()
                c_act = (172 + fd) / 1200.0 / 1000.0
                c_dve = (120 + fd) / 960.0 / 1000.0
                if pick(c_act, c_dve) == 0:
                    nc.scalar.activation(
                        dst, src, COPY, scale=scale if scale is not None else 1.0
                    )
                elif scale is None:
                    nc.vector.tensor_copy(dst, src)
                else:
                    nc.vector.tensor_scalar(
                        out=dst, in0=src, scalar1=scale, scalar2=None, op0=MULT
                    )

            # The host rolls each core's batch sequence so that this core's
            # query rows sit at xT columns 0..SQ-1 (keys are consistently
            # permuted; softmax is permutation-invariant over keys).
            qoff = 0

            # ---------- projections ----------
            with tc.tile_pool(name="ps_proj", bufs=3, space="PSUM") as ps_proj:
                # Q^T: psum slab per head-group g covers dt = 2g, 2g+1
                for g in range(NG):
                    ps = ps_proj.tile([P, 2, SQ], f32)
                    for half in range(2):
                        dtc = 2 * g + half
                        for i in range(4):
                            mm(
                                ps[:, half, :],
                                wq_sb[:, 2 * i : 2 * i + 2, dtc * P : (dtc + 1) * P],
                                xt_sb[:, 2 * i : 2 * i + 2, qoff : qoff + SQ],
                                start=(i == 0), stop=False,
                            )
                        mm(
                            ps[:, half, :],
                            bq_sb[0:1, dtc * P : (dtc + 1) * P],
                            ones_sb[0:1, 0:SQ],
                            start=False, stop=True, dr=False,
                        )
                    evac(qt[:, g, :, :], ps[:])

                # K^T: psum slab per (dt, 1024-key half)
                for dtc in range(DT):
                    g, half = dtc // 2, dtc % 2
                    for kb in range(2):
                        ps = ps_proj.tile([P, 2, SQ], f32)
                        for sub in range(2):
                            koff = kb * 1024 + sub * SQ
                            for i in range(4):
                                mm(
                                    ps[:, sub, :],
                                    wk_sb[:, 2 * i : 2 * i + 2, dtc * P : (dtc + 1) * P],
                                    xt_sb[:, 2 * i : 2 * i + 2, koff : koff + SQ],
                                    start=(i == 0), stop=False,
                                )
                            mm(
                                ps[:, sub, :],
                                bk_sb[0:1, dtc * P : (dtc + 1) * P],
                                ones_sb[0:1, 0:SQ],
                                start=False, stop=True, dr=False,
                            )
                        evac(
                            kt[:, g, half, kb * 1024 : (kb + 1) * 1024],
                            ps[:].rearrange("p a d -> p (a d)"),
                        )

                # V: psum slab per key tile st covers both 512-col halves
                for st in range(KS):
                    ps = ps_proj.tile([P, 2, SQ], f32)
                    for nd in range(2):
                        for i in range(4):
                            mm(
                                ps[:, nd, :],
                                xt_sb[:, 2 * i : 2 * i + 2, st * P : (st + 1) * P],
                                wv_sb[:, 2 * i : 2 * i + 2, nd * SQ : (nd + 1) * SQ],
                                start=(i == 0), stop=False,
                            )
                        mm(
                            ps[:, nd, :],
                            ones_sb[0:1, 0:P],
                            bv_sb[0:1, nd * SQ : (nd + 1) * SQ],
                            start=False, stop=True, dr=False,
                        )
                    evac(
                        v_sb[:, st // 2, st % 2, :, 0:HD],
                        ps[:].rearrange("p a (h c) -> p (a h) c", c=HD),
                        scale=em_sb[:, st : st + 1],
                    )
                    nc.gpsimd.tensor_copy(
                        v_sb[:, st // 2, st % 2, :, HD : HD + 1],
                        em8_sb[:, st : st + 1].to_broadcast((P, H, 1)),
                    )

            # ---------- attention ----------
            def normalize(hb):
                sl = slice(hb * 8, hb * 8 + 8)
                nc.vector.reciprocal(sums_sb[hb][:], sums_sb[hb][:])
                nc.sync.dma_start(sums_dram[sl, :], sums_sb[hb][:])
                for h in range(hb * 8, hb * 8 + 8):
                    t2, off = h // 2, 64 * (h % 2)
                    bcr = bc_pool.tile([HD, SQ], f32, tag="bcr", name="bcr")
                    nc.sync.dma_start(
                        bcr[:],
                        sums_dram[h : h + 1, :].to_broadcast((HD, SQ)),
                    )
                    # even heads: partition-aligned -> GpSimd; odd heads
                    # need a 0->64 partition shift, which only DVE does
                    eng = nc.gpsimd if off == 0 else nc.vector
                    eng.tensor_mul(
                        ctxn[off : off + HD, t2, :], apark[0:HD, h, :], bcr[:]
                    )

            with (
                tc.tile_pool(name="ps_sc", bufs=3, space="PSUM") as ps_sc,
                tc.tile_pool(name="ps_pv", bufs=2, space="PSUM") as ps_pv,
            ):
                for h in range(H):
                    g, r = h // 4, 32 * (h % 4)
                    t2, off = h // 2, 64 * (h % 2)
                    pv = ps_pv.tile([HD + 1, SQ], f32, tag="pv")
                    for u in range(8):
                        sc = ps_sc.tile([P, 2, SQ], f32, tag="sc")
                        for j in range(2):
                            c = 2 * u + j
                            mm(
                                sc[:, j, :],
                                kt[r : r + 32, g, :, c * P : (c + 1) * P],
                                qt[r : r + 32, g, :, :],
                                start=True, stop=True,
                                tile_position=(r, 0),
                            )
                        ex = ex_pool.tile([P, 2, SQ], f8, tag="ex", name="ex")
                        if pick(0.997, 1.192) == 0:
                            nc.scalar.activation(ex[:], sc[:], EXP, scale=ALPHA)
                        else:
                            nc.vector.tensor_scalar(
                                out=ex[:].bitcast(i8), in0=sc[:],
                                scalar1=EXP_A, scalar2=EXP_B,
                                op0=MULT, op1=ADD,
                            )
                        mm(
                            pv[:],
                            v_sb[:, u, :, h, :],
                            ex[:],
                            start=(u == 0), stop=(u == 7),
                        )
                    evac(apark[:, h, :], pv[:])
                    nc.sync.dma_start(
                        sums_sb[h // 8][h % 8 : h % 8 + 1, :],
                        apark[HD : HD + 1, h, :],
                    )
                    if h == 7:
                        normalize(0)
                normalize(1)

            # ---------- O-proj + residual + LayerNorm ----------
            with (
                tc.tile_pool(name="lnconst", bufs=1) as lnc_pool,
                tc.tile_pool(name="ps_o", bufs=2, space="PSUM") as ps_o,
                tc.tile_pool(name="xbuf", bufs=2) as xb_pool,
                tc.tile_pool(name="stats", bufs=4) as st_pool,
            ):
                g_sb = lnc_pool.tile([P, D], f32)
                nc.sync.dma_start(g_sb[:], gamma_bc[:])
                be_sb = lnc_pool.tile([P, D], f32)
                nc.sync.dma_start(be_sb[:], beta_bc[:])
                eps_sb = lnc_pool.tile([P, 1], f32)
                nc.gpsimd.memset(eps_sb[:], EPS)

                for qp in range(4):
                    po = ps_o.tile([P, 2, SQ], f32)
                    for nd in range(2):
                        for i in range(4):
                            mm(
                                po[:, nd, :],
                                ctxn[:, 2 * i : 2 * i + 2, qp * P : (qp + 1) * P],
                                wo_sb[:, 2 * i : 2 * i + 2, nd * SQ : (nd + 1) * SQ],
                                start=(i == 0), stop=(i == 3),
                            )
                    xbuf = xb_pool.tile([P, D], f32)
                    nc.vector.scalar_tensor_tensor(
                        out=xbuf[:],
                        in0=po[:].rearrange("p a d -> p (a d)"),
                        scalar=OSCALE,
                        in1=xq_sb[:, qp, :],
                        op0=MULT, op1=ADD,
                    )
                    stats = st_pool.tile([P, 2, 6], f32)
                    xbuf_v = xbuf[:].rearrange("p (a d) -> p a d", a=2)
                    for a in range(2):
                        nc.vector.bn_stats(stats[:, a, :], xbuf_v[:, a, :])
                    mv = st_pool.tile([P, 2], f32)
                    nc.vector.bn_aggr(mv[:], stats[:])
                    rstd = st_pool.tile([P, 1], f32)
                    nc.scalar.activation(
                        rstd[:],
                        mv[:, 1:2],
                        mybir.ActivationFunctionType.Sqrt,
                        bias=eps_sb[:],
                    )
                    nc.vector.reciprocal(rstd[:], rstd[:])
                    nc.vector.tensor_scalar(
                        out=xbuf[:],
                        in0=xbuf[:],
                        scalar1=mv[:, 0:1],
                        scalar2=rstd[:],
                        op0=mybir.AluOpType.subtract,
                        op1=MULT,
                    )
                    nc.gpsimd.tensor_mul(xbuf[:], xbuf[:], g_sb[:])
                    nc.gpsimd.tensor_add(xbuf[:], xbuf[:], be_sb[:])
                    nc.sync.dma_start(out_r[qp], xbuf[:])

    nc.finalize()
    return nc


def _shard_inputs(inputs):
    """Build the 8 per-core input maps from full inputs."""
    import ml_dtypes

    f8 = ml_dtypes.float8_e4m3
    bf = ml_dtypes.bfloat16

    x = np.ascontiguousarray(np.asarray(inputs["hidden_states"], dtype=np.float32))
    mask = np.asarray(inputs["attention_mask"], dtype=np.float32).reshape(B, S)
    perm = _perm()

    def w8(name, permute=False):
        w = np.asarray(inputs[name], dtype=np.float32) * WSCALE
        if permute:
            w = w[:, perm]
        return np.ascontiguousarray(np.clip(w, -F8MAX, F8MAX).astype(f8))

    Wq8, Wk8 = w8("Wq", True), w8("Wk", True)
    Wv8, Wo8 = w8("Wv"), w8("Wo")
    bq = np.asarray(inputs["bq"], dtype=np.float32)
    bk = np.asarray(inputs["bk"], dtype=np.float32)
    bv = np.asarray(inputs["bv"], dtype=np.float32)
    bo = np.asarray(inputs["bo"], dtype=np.float32)
    bq_r = np.ascontiguousarray((WSCALE * bq[perm]).reshape(1, D).astype(bf))
    bk_r = np.ascontiguousarray((WSCALE * bk[perm]).reshape(1, D).astype(bf))
    bv_r = np.ascontiguousarray((WSCALE * bv).reshape(1, D).astype(bf))
    gamma = np.asarray(inputs["ln_gamma"], dtype=np.float32)
    beta = np.asarray(inputs["ln_beta"], dtype=np.float32)
    gamma_bc = np.ascontiguousarray(np.broadcast_to(gamma, (P, D)))
    beta_bc = np.ascontiguousarray(np.broadcast_to(beta, (P, D)))

    em = np.exp(mask)  # [B, S]

    in_maps = []
    for c in range(NCORES):
        b, q = c // 4, (c % 4) * SQ
        # roll the batch's sequence so this core's queries are cols 0..SQ-1
        xb = np.roll(x[b], -q, axis=0)  # [S, D]
        emb = np.roll(em[b], -q)
        xT8 = np.ascontiguousarray(
            np.clip(xb.T, -F8MAX, F8MAX).astype(f8)
        )
        em_t = np.ascontiguousarray(emb.reshape(KS, P).T)
        in_maps.append(
            {
                "xT": xT8,
                "xq": np.ascontiguousarray(x[b, q : q + SQ, :] + bo),
                "Wq": Wq8, "Wk": Wk8, "Wv": Wv8, "Wo": Wo8,
                "bq_r": bq_r, "bk_r": bk_r, "bv_r": bv_r,
                "em_t": em_t,
                "em8_t": np.ascontiguousarray(em_t * DCOL),
                "gamma_bc": gamma_bc, "beta_bc": beta_bc,
            }
        )
    return in_maps


def run(inputs, trace=False, **kw):
    """Run on hardware; returns (full_output, BassKernelResults)."""
    _ensure_paths()
    from concourse.bass_utils import run_bass_kernel_spmd

    if "nc" not in _CACHE:
        _CACHE["nc"] = build_nc()
    nc = _CACHE["nc"]
    in_maps = _shard_inputs(inputs)
    res = run_bass_kernel_spmd(
        nc, in_maps, core_ids=list(range(NCORES)), trace=trace, **kw
    )
    parts = [res.results[c]["out"] for c in range(NCORES)]
    full = np.empty((B, S, D), dtype=np.float32)
    for c in range(NCORES):
        b, q = c // 4, (c % 4) * SQ
        full[b, q : q + SQ] = parts[c]
    return full, res


def kernel(**inputs):
    out, _ = run(inputs)
    return out


# revision 15
# speedup vs baseline: 1.2240x; 1.2240x over previous
"""BertAttention (B=2,S=2048,D=1024,H=16) on 8 trn2 NeuronCores — v3.

Sharding: data-parallel over B (2 groups of 4 cores); each group's 4 cores
split the 2048 query rows (512 each). Every core computes K^T and V for its
batch in full (redundant within the group), its own 512-row Q slice,
attention over all 16 heads for its rows, output projection, residual and
LayerNorm. Each core emits a disjoint [512, 1024] output slice.

PE strategy (micro-benchmarked):
  - projections (Q/K/V/O) in fp8 DoubleRow — 2 k-tiles per matmul, 1.94x.
  - scores in plain fp8 with TWO HEADS ROW-TILED concurrently in the PE
    array (tile_position (0,0)/(64,0)) — 108 ns per 512-free matmul.
  - PV swapped: lhsT = exp-chunk [128keys x 128q], rhs = V [128keys x 66]
    (64 ctx dims + denominator col + pad) — 66-cycle matmuls, ~31 ns.
    ctx lands q-major so the softmax denominator is a per-partition
    column: one tiny reciprocal + one broadcast multiply per head.
    PE transposes (fp8) restore d-major ctx for the O-projection.
  - QKV biases: K/Q bias is a per-partition column folded into the
    PSUM->SBUF evacuation (Identity activation with bias AP on ACT,
    tensor_scalar on DVE); V bias enters via rank-1 bias matmuls.
exp is split across ACT (native Exp -> fp8) and DVE (Schraudolph int8
bit-trick), greedily balanced by modeled cost.
"""

import numpy as np

B, S, D, H = 2, 2048, 1024, 16
HD = D // H  # 64
P = 128
NCORES = 8
SQ = S // 4  # 512 query rows per core
DT = D // P  # 8 feature tiles
KS = S // P  # 16 key tiles
NT2 = H // 2  # 8 head pairs
VW = HD + 2  # 66: V row = 64 ctx dims + denom col + pad
EPS = 1e-12

WSCALE = 32.0  # weight pre-scale (fp8 denormal avoidance)
F8MAX = 240.0  # TRN fp8e4 max normal (above: Inf!)
ALPHA = 1.0 / (WSCALE * WSCALE * 8.0)  # exp scale on raw score psum (2^-13)
EXP_A = 8.0 * 1.4426950408889634 * ALPHA  # DVE bit-trick multiplier
EXP_B = 56.344  # DVE bit-trick bias
CTX_S = 256.0  # ctxq = CTX_S * ctx
DCOL = WSCALE / CTX_S  # 1/8: V denominator-column scale
OSCALE = 1.0 / (WSCALE * CTX_S)  # 1/8192: O-proj psum descale

_CACHE = {}


def _ensure_paths():
    try:
        import concourse  # noqa: F401
    except ImportError:
        import sys

        for p in ("/opt/trn_rl_repo", "/root/.axon_site/_ro/trn_rl_repo"):
            if p not in sys.path:
                sys.path.append(p)
        import concourse  # noqa: F401


def build_nc():
    _ensure_paths()
    import concourse.tile as tile
    from concourse import bacc, mybir

    f32 = mybir.dt.float32
    bf16 = mybir.dt.bfloat16
    f8 = mybir.dt.float8e4
    i8 = mybir.dt.int8
    DRM = mybir.MatmulPerfMode.DoubleRow
    EXP = mybir.ActivationFunctionType.Exp
    IDENT = mybir.ActivationFunctionType.Identity
    COPY = mybir.ActivationFunctionType.Copy
    MULT = mybir.AluOpType.mult
    ADD = mybir.AluOpType.add

    nc = bacc.Bacc()

    # ---- I/O ----
    xT = nc.declare_dram_parameter("xT", [D, S], f8, isOutput=False)
    xq = nc.declare_dram_parameter("xq", [SQ, D], f32, isOutput=False)
    Wq = nc.declare_dram_parameter("Wq", [D, D], f8, isOutput=False)
    Wk = nc.declare_dram_parameter("Wk", [D, D], f8, isOutput=False)
    Wv = nc.declare_dram_parameter("Wv", [D, D], f8, isOutput=False)
    Wo = nc.declare_dram_parameter("Wo", [D, D], f8, isOutput=False)
    bq_c = nc.declare_dram_parameter("bq_c", [P, DT], f32, isOutput=False)
    bk_c = nc.declare_dram_parameter("bk_c", [P, DT], f32, isOutput=False)
    bv_r = nc.declare_dram_parameter("bv_r", [1, D], bf16, isOutput=False)
    em_t = nc.declare_dram_parameter("em_t", [P, KS], f32, isOutput=False)
    ident = nc.declare_dram_parameter("ident", [P, P], f8, isOutput=False)
    gamma_bc = nc.declare_dram_parameter("gamma_bc", [P, D], f32, isOutput=False)
    beta_bc = nc.declare_dram_parameter("beta_bc", [P, D], f32, isOutput=False)
    out = nc.declare_dram_parameter("out", [SQ, D], f32, isOutput=True)

    xT_r = xT.rearrange("(t p) s -> p t s", p=P)
    W_r = {
        "q": Wq.rearrange("(t p) d -> p t d", p=P),
        "k": Wk.rearrange("(t p) d -> p t d", p=P),
        "v": Wv.rearrange("(t p) d -> p t d", p=P),
        "o": Wo.rearrange("(t p) d -> p t d", p=P),
    }
    xq_r = xq.rearrange("(t p) d -> p t d", p=P)
    out_r = out.rearrange("(t p) d -> t p d", p=P)

    def mm(ps, lhsT, rhs, start, stop, dr=False, tile_position=None):
        nc.tensor.matmul(
            ps, lhsT, rhs, start=start, stop=stop,
            perf_mode=DRM if dr else None,
            tile_position=tile_position,
        )

    # greedy ACT/DVE balancing (modeled op cost in µs); DVE preloaded with
    # its exclusive late work (ctxq evacs, recips, STT, bn, LN)
    eng_t = [0.0, 20.0]

    def pick(cost_act, cost_dve):
        if eng_t[0] + cost_act <= eng_t[1] + cost_dve:
            eng_t[0] += cost_act
            return 0
        eng_t[1] += cost_dve
        return 1

    with tile.TileContext(nc) as tc:
        with (
            tc.tile_pool(name="consts", bufs=1) as consts,
            tc.tile_pool(name="pers", bufs=1) as pers,
            tc.tile_pool(name="exp", bufs=6) as ex_pool,
        ):
            # ---- inputs ----
            xt_sb = pers.tile([P, DT, S], f8)
            nc.sync.dma_start(xt_sb[:], xT_r[:])
            wq_sb = pers.tile([P, DT, D], f8)
            nc.sync.dma_start(wq_sb[:], W_r["q"][:])
            wk_sb = pers.tile([P, DT, D], f8)
            nc.sync.dma_start(wk_sb[:], W_r["k"][:])
            wv_sb = pers.tile([P, DT, D], f8)
            nc.sync.dma_start(wv_sb[:], W_r["v"][:])

            ones_sb = consts.tile([1, D], bf16)
            nc.gpsimd.memset(ones_sb[:], 1.0)
            bq_sb = consts.tile([P, DT], f32)
            nc.sync.dma_start(bq_sb[:], bq_c[:])
            bk_sb = consts.tile([P, DT], f32)
            nc.sync.dma_start(bk_sb[:], bk_c[:])
            bv_sb = consts.tile([1, D], bf16)
            nc.sync.dma_start(bv_sb[:], bv_r[:])
            em_sb = consts.tile([P, KS], f32)
            nc.sync.dma_start(em_sb[:], em_t[:])
            id_sb = consts.tile([P, P], f8)
            nc.sync.dma_start(id_sb[:], ident[:])

            # persistent activation tiles (baseline-style head layout:
            # head pair t2 = h//2, head h at partitions 64*(h%2))
            qt = pers.tile([P, NT2, SQ], f8)  # Q^T
            kt = pers.tile([P, NT2, S], f8)  # K^T
            v_sb = pers.tile([P, KS, H, VW], f8)  # V rows + denom col
            ctxq = pers.tile([P, 4, D], f8)  # q-major normalized ctx
            ctxn = pers.tile([P, DT, SQ], f8)  # d-major normalized ctx^T
            xq_sb = pers.tile([P, 4, D], f32)
            nc.sync.dma_start(xq_sb[:], xq_r[:])
            wo_sb = pers.tile([P, DT, D], f8)
            nc.sync.dma_start(wo_sb[:], W_r["o"][:])

            # V denominator column: em/8 per key; pad col zeroed
            for st in range(KS):
                nc.gpsimd.tensor_scalar(
                    out=v_sb[:, st, :, HD : HD + 1],
                    in0=em_sb[:, st : st + 1].to_broadcast((P, H, 1)),
                    scalar1=DCOL, scalar2=None, op0=MULT,
                )
            nc.gpsimd.memset(v_sb[:, :, :, HD + 1 : HD + 2], 0.0)

            def evac(dst, src, scale=None, bias=None):
                """PSUM->SBUF convert-copy on ACT or DVE (greedy).
                scale: per-partition column AP or None; bias: column AP."""
                fd = src.free_size()
                c_act = (172 + fd) / 1200.0 / 1000.0
                c_dve = (120 + fd) / 960.0 / 1000.0
                if pick(c_act, c_dve) == 0:
                    if bias is None:
                        nc.scalar.activation(
                            dst, src, COPY,
                            scale=scale if scale is not None else 1.0,
                        )
                    else:
                        nc.scalar.activation(dst, src, IDENT, bias=bias)
                elif scale is None and bias is None:
                    nc.vector.tensor_copy(dst, src)
                else:
                    s1 = scale if scale is not None else bias
                    op = MULT if scale is not None else ADD
                    nc.vector.tensor_scalar(
                        out=dst, in0=src, scalar1=s1, scalar2=None, op0=op
                    )

            # ---------- projections ----------
            with tc.tile_pool(name="ps_proj", bufs=3, space="PSUM") as ps_proj:
                # Q^T: per t2, psum [128, 512]
                for t2 in range(NT2):
                    ps = ps_proj.tile([P, 2, SQ], f32, tag="pp")
                    for i in range(4):
                        mm(
                            ps[:, 0, :],
                            wq_sb[:, 2 * i : 2 * i + 2, t2 * P : (t2 + 1) * P],
                            xt_sb[:, 2 * i : 2 * i + 2, 0:SQ],
                            start=(i == 0), stop=(i == 3), dr=True,
                        )
                    evac(qt[:, t2, :], ps[:, 0, :], bias=bq_sb[:, t2 : t2 + 1])

                # K^T: per (t2, kb of 1024 keys): psum [128, 2, 512]
                for t2 in range(NT2):
                    for kb in range(2):
                        ps = ps_proj.tile([P, 2, SQ], f32, tag="pp")
                        for sub in range(2):
                            koff = kb * 1024 + sub * SQ
                            for i in range(4):
                                mm(
                                    ps[:, sub, :],
                                    wk_sb[:, 2 * i : 2 * i + 2, t2 * P : (t2 + 1) * P],
                                    xt_sb[:, 2 * i : 2 * i + 2, koff : koff + SQ],
                                    start=(i == 0), stop=(i == 3), dr=True,
                                )
                        evac(
                            kt[:, t2, kb * 1024 : (kb + 1) * 1024],
                            ps[:].rearrange("p a d -> p (a d)"),
                            bias=bk_sb[:, t2 : t2 + 1],
                        )

                # V: per key tile st: psum [128, 2, 512] (vd halves)
                for st in range(KS):
                    ps = ps_proj.tile([P, 2, SQ], f32, tag="pp")
                    for nd in range(2):
                        for i in range(4):
                            mm(
                                ps[:, nd, :],
                                xt_sb[:, 2 * i : 2 * i + 2, st * P : (st + 1) * P],
                                wv_sb[:, 2 * i : 2 * i + 2, nd * SQ : (nd + 1) * SQ],
                                start=(i == 0), stop=False, dr=True,
                            )
                        mm(
                            ps[:, nd, :],
                            ones_sb[0:1, 0:P],
                            bv_sb[0:1, nd * SQ : (nd + 1) * SQ],
                            start=False, stop=True,
                        )
                    evac(
                        v_sb[:, st, :, 0:HD],
                        ps[:].rearrange("p a (h c) -> p (a h) c", c=HD),
                        scale=em_sb[:, st : st + 1],
                    )

            # ---------- attention (head pairs, row-tiled scores) ----------
            # software-pipelined: PV of unit u-1 runs while exp of unit u
            # is in flight, so the PE never waits on the activation engines
            with (
                tc.tile_pool(name="ps_sc", bufs=3, space="PSUM") as ps_sc,
                tc.tile_pool(name="ps_pv", bufs=1, space="PSUM") as ps_pv,
                tc.tile_pool(name="rcp", bufs=2) as rcp_pool,
            ):
                for t2 in range(NT2):
                    hA, hB = 2 * t2, 2 * t2 + 1
                    pvA = ps_pv.tile([P, 4, VW], f32, tag="pvA", name="pvA")
                    pvB = ps_pv.tile([P, 4, VW], f32, tag="pvB", name="pvB")
                    heads = ((hA, 0, pvA), (hB, 64, pvB))
                    exs = {}

                    def emit_pv(u):
                        for hi, (h, _, pv) in enumerate(heads):
                            ex = exs.pop((u, hi))
                            for j in range(2):
                                c = 2 * u + j
                                for qc in range(4):
                                    mm(
                                        pv[:, qc, :],
                                        ex[:, j, qc * P : (qc + 1) * P],
                                        v_sb[:, c, h, :],
                                        start=(u == 0 and j == 0),
                                        stop=(u == 7 and j == 1),
                                    )

                    for u in range(8):  # units of 2 key tiles
                        for hi, (h, roff, pv) in enumerate(heads):
                            sc = ps_sc.tile([P, 2, SQ], f32, tag="sc")
                            for j in range(2):
                                c = 2 * u + j
                                mm(
                                    sc[:, j, :],
                                    kt[roff : roff + HD, t2, c * P : (c + 1) * P],
                                    qt[roff : roff + HD, t2, :],
                                    start=True, stop=True,
                                    tile_position=(roff, 0),
                                )
                            ex = ex_pool.tile([P, 2, SQ], f8, tag="ex", name="ex")
                            if pick(0.997, 1.192) == 0:
                                nc.scalar.activation(ex[:], sc[:], EXP, scale=ALPHA)
                            else:
                                nc.vector.tensor_scalar(
                                    out=ex[:].bitcast(i8), in0=sc[:],
                                    scalar1=EXP_A, scalar2=EXP_B,
                                    op0=MULT, op1=ADD,
                                )
                            exs[(u, hi)] = ex
                        if u > 0:
                            emit_pv(u - 1)
                    emit_pv(7)
                    # normalize: denom col 64 -> reciprocal -> broadcast mul
                    for h, _, pv in heads:
                        rcp = rcp_pool.tile([P, 4, 1], f32, tag="rcp", name="rcp")
                        nc.vector.reciprocal(rcp[:], pv[:, :, HD : HD + 1])
                        nc.vector.tensor_mul(
                            ctxq[:, :, h * HD : (h + 1) * HD],
                            pv[:, :, 0:HD],
                            rcp[:].to_broadcast((P, 4, HD)),
                        )

            # ---------- ctx reorientation (q-major -> d-major) ----------
            with tc.tile_pool(name="ps_t", bufs=4, space="PSUM") as ps_t:
                for qc in range(4):
                    for t2 in range(NT2):
                        # fp8 transpose writes with element step 2
                        pt = ps_t.tile([P, P, 2], f8, tag="pt", name="pt")
                        nc.tensor.transpose(
                            pt[:, :, 0], ctxq[:, qc, t2 * P : (t2 + 1) * P], id_sb[:]
                        )
                        evac(ctxn[:, t2, qc * P : (qc + 1) * P], pt[:, :, 0])

            # ---------- O-proj + residual + LayerNorm ----------
            with (
                tc.tile_pool(name="lnconst", bufs=1) as lnc_pool,
                tc.tile_pool(name="ps_o", bufs=2, space="PSUM") as ps_o,
                tc.tile_pool(name="xbuf", bufs=2) as xb_pool,
                tc.tile_pool(name="stats", bufs=4) as st_pool,
            ):
                g_sb = lnc_pool.tile([P, D], f32)
                nc.sync.dma_start(g_sb[:], gamma_bc[:])
                be_sb = lnc_pool.tile([P, D], f32)
                nc.sync.dma_start(be_sb[:], beta_bc[:])
                eps_sb = lnc_pool.tile([P, 1], f32)
                nc.gpsimd.memset(eps_sb[:], EPS)

                for qp in range(4):
                    po = ps_o.tile([P, 2, SQ], f32)
                    for nd in range(2):
                        for i in range(4):
                            mm(
                                po[:, nd, :],
                                ctxn[:, 2 * i : 2 * i + 2, qp * P : (qp + 1) * P],
                                wo_sb[:, 2 * i : 2 * i + 2, nd * SQ : (nd + 1) * SQ],
                                start=(i == 0), stop=(i == 3), dr=True,
                            )
                    xbuf = xb_pool.tile([P, D], f32)
                    nc.vector.scalar_tensor_tensor(
                        out=xbuf[:],
                        in0=po[:].rearrange("p a d -> p (a d)"),
                        scalar=OSCALE,
                        in1=xq_sb[:, qp, :],
                        op0=MULT, op1=ADD,
                    )
                    stats = st_pool.tile([P, 2, 6], f32)
                    xbuf_v = xbuf[:].rearrange("p (a d) -> p a d", a=2)
                    for a in range(2):
                        nc.vector.bn_stats(stats[:, a, :], xbuf_v[:, a, :])
                    mv = st_pool.tile([P, 2], f32)
                    nc.vector.bn_aggr(mv[:], stats[:])
                    rstd = st_pool.tile([P, 1], f32)
                    nc.scalar.activation(
                        rstd[:],
                        mv[:, 1:2],
                        mybir.ActivationFunctionType.Sqrt,
                        bias=eps_sb[:],
                    )
                    nc.vector.reciprocal(rstd[:], rstd[:])
                    nc.vector.tensor_scalar(
                        out=xbuf[:],
                        in0=xbuf[:],
                        scalar1=mv[:, 0:1],
                        scalar2=rstd[:],
                        op0=mybir.AluOpType.subtract,
                        op1=MULT,
                    )
                    nc.gpsimd.tensor_mul(xbuf[:], xbuf[:], g_sb[:])
                    nc.gpsimd.tensor_add(xbuf[:], xbuf[:], be_sb[:])
                    nc.sync.dma_start(out_r[qp], xbuf[:])

    nc.finalize()
    return nc


def _shard_inputs(inputs):
    """Build the 8 per-core input maps from full inputs."""
    import ml_dtypes

    f8 = ml_dtypes.float8_e4m3
    bf = ml_dtypes.bfloat16

    x = np.ascontiguousarray(np.asarray(inputs["hidden_states"], dtype=np.float32))
    mask = np.asarray(inputs["attention_mask"], dtype=np.float32).reshape(B, S)

    def w8(name):
        w = np.asarray(inputs[name], dtype=np.float32) * WSCALE
        return np.ascontiguousarray(np.clip(w, -F8MAX, F8MAX).astype(f8))

    Wq8, Wk8, Wv8, Wo8 = w8("Wq"), w8("Wk"), w8("Wv"), w8("Wo")
    bq = np.asarray(inputs["bq"], dtype=np.float32)
    bk = np.asarray(inputs["bk"], dtype=np.float32)
    bv = np.asarray(inputs["bv"], dtype=np.float32)
    bo = np.asarray(inputs["bo"], dtype=np.float32)
    # K/Q bias as per-partition columns matching the psum slab layout:
    # slab t2 partition p holds feature d = t2*128 + p
    bq_c = np.ascontiguousarray((WSCALE * bq).reshape(DT, P).T)
    bk_c = np.ascontiguousarray((WSCALE * bk).reshape(DT, P).T)
    bv_r = np.ascontiguousarray((WSCALE * bv).reshape(1, D).astype(bf))
    gamma = np.asarray(inputs["ln_gamma"], dtype=np.float32)
    beta = np.asarray(inputs["ln_beta"], dtype=np.float32)
    gamma_bc = np.ascontiguousarray(np.broadcast_to(gamma, (P, D)))
    beta_bc = np.ascontiguousarray(np.broadcast_to(beta, (P, D)))
    ident = np.ascontiguousarray(np.eye(P, dtype=np.float32).astype(f8))

    em = np.exp(mask)  # [B, S]

    in_maps = []
    for c in range(NCORES):
        b, q = c // 4, (c % 4) * SQ
        # roll the batch's sequence so this core's queries are cols 0..SQ-1
        xb = np.roll(x[b], -q, axis=0)  # [S, D]
        emb = np.roll(em[b], -q)
        xT8 = np.ascontiguousarray(np.clip(xb.T, -F8MAX, F8MAX).astype(f8))
        em_t = np.ascontiguousarray(emb.reshape(KS, P).T)
        in_maps.append(
            {
                "xT": xT8,
                "xq": np.ascontiguousarray(x[b, q : q + SQ, :] + bo),
                "Wq": Wq8, "Wk": Wk8, "Wv": Wv8, "Wo": Wo8,
                "bq_c": bq_c, "bk_c": bk_c, "bv_r": bv_r,
                "em_t": em_t,
                "ident": ident,
                "gamma_bc": gamma_bc, "beta_bc": beta_bc,
            }
        )
    return in_maps


def run(inputs, trace=False, **kw):
    """Run on hardware; returns (full_output, BassKernelResults)."""
    _ensure_paths()
    from concourse.bass_utils import run_bass_kernel_spmd

    if "nc" not in _CACHE:
        _CACHE["nc"] = build_nc()
    nc = _CACHE["nc"]
    in_maps = _shard_inputs(inputs)
    res = run_bass_kernel_spmd(
        nc, in_maps, core_ids=list(range(NCORES)), trace=trace, **kw
    )
    parts = [res.results[c]["out"] for c in range(NCORES)]
    full = np.empty((B, S, D), dtype=np.float32)
    for c in range(NCORES):
        b, q = c // 4, (c % 4) * SQ
        full[b, q : q + SQ] = parts[c]
    return full, res


def kernel(**inputs):
    out, _ = run(inputs)
    return out


# revision 18
# speedup vs baseline: 1.2461x; 1.0181x over previous
"""BertAttention (B=2,S=2048,D=1024,H=16) on 8 trn2 NeuronCores — v3.2.

Sharding: data-parallel over B (2 groups of 4 cores); each group's 4 cores
split the 2048 query rows (512 each). Every core computes K^T and V for its
batch in full (redundant within the group), its own 512-row Q slice,
attention over all 16 heads for its rows, output projection, residual and
LayerNorm. Each core emits a disjoint [512, 1024] output slice.

The attention phase is bound by the exp throughput of ACT+DVE (GpSimd has
no PSUM port, and DMA cannot read PSUM, so only those two engines can
consume score PSUM). The kernel therefore FUSES the projection work into
the attention stream: a prelude computes Q(t2=0), K(t2=0) and most of V,
and the remaining V/K/Q projection items are emitted as PE fillers between
attention units, soaking up PE idle while ACT/DVE chew exp.

PE patterns (micro-benchmarked):
  - projections in fp8 DoubleRow (2 k-tiles per matmul, ~1.9x)
  - scores plain fp8, two heads row-tiled concurrently ((0,0)/(64,0))
  - PV in fp8 DoubleRow over key-tile pairs: lhsT = exp [128,2,128q],
    rhs = V [128,2,66] -> ~40ns per matmul, ctx lands q-major so softmax
    normalization is a per-partition reciprocal + one broadcast multiply
  - PE transposes restore d-major ctx for the O-projection
exp: ACT native Exp->fp8 and DVE Schraudolph int8 bit-trick, greedily
balanced with measured costs; PV trails exp by 2 units so the PE never
waits on the activation engines.
"""

import numpy as np

B, S, D, H = 2, 2048, 1024, 16
HD = D // H  # 64
P = 128
NCORES = 8
SQ = S // 4  # 512
DT = D // P  # 8
KS = S // P  # 16
NT2 = H // 2  # 8
VW = HD + 2  # 66
EPS = 1e-12

WSCALE = 32.0
F8MAX = 240.0
ALPHA = 1.0 / (WSCALE * WSCALE * 8.0)
EXP_A = 8.0 * 1.4426950408889634 * ALPHA
EXP_B = 56.344
CTX_S = 256.0
DCOL = WSCALE / CTX_S
OSCALE = 1.0 / (WSCALE * CTX_S)

_CACHE = {}


def _ensure_paths():
    try:
        import concourse  # noqa: F401
    except ImportError:
        import sys

        for p in ("/opt/trn_rl_repo", "/root/.axon_site/_ro/trn_rl_repo"):
            if p not in sys.path:
                sys.path.append(p)
        import concourse  # noqa: F401


def build_nc():
    _ensure_paths()
    import concourse.tile as tile
    from concourse import bacc, mybir

    f32 = mybir.dt.float32
    bf16 = mybir.dt.bfloat16
    f8 = mybir.dt.float8e4
    i8 = mybir.dt.int8
    DRM = mybir.MatmulPerfMode.DoubleRow
    EXP = mybir.ActivationFunctionType.Exp
    IDENT = mybir.ActivationFunctionType.Identity
    COPY = mybir.ActivationFunctionType.Copy
    MULT = mybir.AluOpType.mult
    ADD = mybir.AluOpType.add

    nc = bacc.Bacc()

    xT = nc.declare_dram_parameter("xT", [D, S], f8, isOutput=False)
    xq = nc.declare_dram_parameter("xq", [SQ, D], f32, isOutput=False)
    Wq = nc.declare_dram_parameter("Wq", [D, D], f8, isOutput=False)
    Wk = nc.declare_dram_parameter("Wk", [D, D], f8, isOutput=False)
    Wv = nc.declare_dram_parameter("Wv", [D, D], f8, isOutput=False)
    Wo = nc.declare_dram_parameter("Wo", [D, D], f8, isOutput=False)
    bq_c = nc.declare_dram_parameter("bq_c", [P, DT], f32, isOutput=False)
    bk_c = nc.declare_dram_parameter("bk_c", [P, DT], f32, isOutput=False)
    bv_r = nc.declare_dram_parameter("bv_r", [1, D], bf16, isOutput=False)
    em_t = nc.declare_dram_parameter("em_t", [P, KS], f32, isOutput=False)
    ident = nc.declare_dram_parameter("ident", [P, P], f8, isOutput=False)
    gamma_bc = nc.declare_dram_parameter("gamma_bc", [P, D], f32, isOutput=False)
    beta_bc = nc.declare_dram_parameter("beta_bc", [P, D], f32, isOutput=False)
    out = nc.declare_dram_parameter("out", [SQ, D], f32, isOutput=True)

    xT_r = xT.rearrange("(t p) s -> p t s", p=P)
    W_r = {
        "q": Wq.rearrange("(t p) d -> p t d", p=P),
        "k": Wk.rearrange("(t p) d -> p t d", p=P),
        "v": Wv.rearrange("(t p) d -> p t d", p=P),
        "o": Wo.rearrange("(t p) d -> p t d", p=P),
    }
    xq_r = xq.rearrange("(t p) d -> p t d", p=P)
    out_r = out.rearrange("(t p) d -> t p d", p=P)

    def mm(ps, lhsT, rhs, start, stop, dr=False, tile_position=None):
        nc.tensor.matmul(
            ps, lhsT, rhs, start=start, stop=stop,
            perf_mode=DRM if dr else None,
            tile_position=tile_position,
        )

    # measured per-op engine costs (µs): linear in free-dim
    def c_act(fd):
        return 1.098e-3 * (172 + fd)

    def c_dve(fd):
        return 1.2215e-3 * (120 + fd)

    eng_t = [0.0, 27.0]  # DVE preloaded with its exclusive late work

    def pick(ca, cd):
        if eng_t[0] + ca <= eng_t[1] + cd:
            eng_t[0] += ca
            return 0
        eng_t[1] += cd
        return 1

    with tile.TileContext(nc) as tc:
        with (
            tc.tile_pool(name="consts", bufs=1) as consts,
            tc.tile_pool(name="pers", bufs=1) as pers,
            tc.tile_pool(name="exp", bufs=8) as ex_pool,
            tc.tile_pool(name="rcp", bufs=2) as rcp_pool,
            tc.tile_pool(name="pw", bufs=2, space="PSUM") as pw_pool,
            tc.tile_pool(name="ps_sc", bufs=2, space="PSUM") as ps_sc,
            tc.tile_pool(name="ps_pv", bufs=1, space="PSUM") as ps_pv,
        ):
            # ---- inputs; xT in 4 column chunks so compute starts early
            xt_sb = pers.tile([P, DT, S], f8)
            for ch in range(4):
                nc.sync.dma_start(
                    xt_sb[:, :, ch * SQ : (ch + 1) * SQ],
                    xT_r[:, :, ch * SQ : (ch + 1) * SQ],
                )
            wq_sb = pers.tile([P, DT, D], f8)
            nc.sync.dma_start(wq_sb[:], W_r["q"][:])
            wk_sb = pers.tile([P, DT, D], f8)
            nc.sync.dma_start(wk_sb[:], W_r["k"][:])
            wv_sb = pers.tile([P, DT, D], f8)
            nc.sync.dma_start(wv_sb[:], W_r["v"][:])

            ones_sb = consts.tile([1, D], bf16)
            nc.gpsimd.memset(ones_sb[:], 1.0)
            bq_sb = consts.tile([P, DT], f32)
            nc.sync.dma_start(bq_sb[:], bq_c[:])
            bk_sb = consts.tile([P, DT], f32)
            nc.sync.dma_start(bk_sb[:], bk_c[:])
            bv_sb = consts.tile([1, D], bf16)
            nc.sync.dma_start(bv_sb[:], bv_r[:])
            em_sb = consts.tile([P, KS], f32)
            nc.sync.dma_start(em_sb[:], em_t[:])
            id_sb = consts.tile([P, P], f8)
            nc.sync.dma_start(id_sb[:], ident[:])

            qt = pers.tile([P, NT2, SQ], f8)
            kt = pers.tile([P, NT2, S], f8)
            v_sb = pers.tile([P, KS, H, VW], f8)
            ctxq = pers.tile([P, 4, D], f8)
            ctxn = pers.tile([P, DT, SQ], f8)
            xq_sb = pers.tile([P, 4, D], f32)
            nc.sync.dma_start(xq_sb[:], xq_r[:])
            wo_sb = pers.tile([P, DT, D], f8)
            nc.sync.dma_start(wo_sb[:], W_r["o"][:])

            for st in range(KS):
                nc.gpsimd.tensor_scalar(
                    out=v_sb[:, st, :, HD : HD + 1],
                    in0=em_sb[:, st : st + 1].to_broadcast((P, H, 1)),
                    scalar1=DCOL, scalar2=None, op0=MULT,
                )
            nc.gpsimd.memset(v_sb[:, :, :, HD + 1 : HD + 2], 0.0)

            def evac(dst, src, scale=None, bias=None):
                fd = src.free_size()
                if pick(c_act(fd), c_dve(fd)) == 0:
                    if bias is None:
                        nc.scalar.activation(
                            dst, src, COPY,
                            scale=scale if scale is not None else 1.0,
                        )
                    else:
                        nc.scalar.activation(dst, src, IDENT, bias=bias)
                elif scale is None and bias is None:
                    nc.vector.tensor_copy(dst, src)
                else:
                    s1 = scale if scale is not None else bias
                    op = MULT if scale is not None else ADD
                    nc.vector.tensor_scalar(
                        out=dst, in0=src, scalar1=s1, scalar2=None, op0=op
                    )

            # ---- projection items (fillers), all on 1-bank [P,512] psums
            def q_item(t2):
                ps = pw_pool.tile([P, SQ], f32, tag="pw", name="pw")
                for i in range(4):
                    mm(
                        ps[:],
                        wq_sb[:, 2 * i : 2 * i + 2, t2 * P : (t2 + 1) * P],
                        xt_sb[:, 2 * i : 2 * i + 2, 0:SQ],
                        start=(i == 0), stop=(i == 3), dr=True,
                    )
                evac(qt[:, t2, :], ps[:], bias=bq_sb[:, t2 : t2 + 1])

            def k_item(t2, kq):  # kq in 0..3: 512-key block
                ps = pw_pool.tile([P, SQ], f32, tag="pw", name="pw")
                for i in range(4):
                    mm(
                        ps[:],
                        wk_sb[:, 2 * i : 2 * i + 2, t2 * P : (t2 + 1) * P],
                        xt_sb[:, 2 * i : 2 * i + 2, kq * SQ : (kq + 1) * SQ],
                        start=(i == 0), stop=(i == 3), dr=True,
                    )
                evac(
                    kt[:, t2, kq * SQ : (kq + 1) * SQ],
                    ps[:],
                    bias=bk_sb[:, t2 : t2 + 1],
                )

            def v_item(st, nd):
                ps = pw_pool.tile([P, SQ], f32, tag="pw", name="pw")
                for i in range(4):
                    mm(
                        ps[:],
                        xt_sb[:, 2 * i : 2 * i + 2, st * P : (st + 1) * P],
                        wv_sb[:, 2 * i : 2 * i + 2, nd * SQ : (nd + 1) * SQ],
                        start=(i == 0), stop=False, dr=True,
                    )
                mm(
                    ps[:],
                    ones_sb[0:1, 0:P],
                    bv_sb[0:1, nd * SQ : (nd + 1) * SQ],
                    start=False, stop=True,
                )
                evac(
                    v_sb[:, st, nd * 8 : (nd + 1) * 8, 0:HD],
                    ps[:].rearrange("p (h c) -> p h c", c=HD),
                    scale=em_sb[:, st : st + 1],
                )

            # filler schedule: prelude covers pair-0's needs; the rest of
            # the projections interleave into the attention stream
            prelude = [lambda: q_item(0)]
            prelude += [lambda kq=kq: k_item(0, kq) for kq in range(4)]
            for st in range(12):
                prelude += [lambda s=st, nd=nd: v_item(s, nd) for nd in range(2)]
            fillers = {t2: [] for t2 in range(NT2)}
            for st in range(12, KS):  # V12..15 early in pair 0
                fillers[0] += [lambda s=st, nd=nd: v_item(s, nd) for nd in range(2)]
            for t2 in range(NT2 - 1):  # Q/K for pair t2+1 during pair t2
                fillers[t2].append(lambda n=t2 + 1: q_item(n))
                fillers[t2] += [
                    lambda n=t2 + 1, kq=kq: k_item(n, kq) for kq in range(4)
                ]

            for f in prelude:
                f()

            # ---------- fused attention ----------
            for t2 in range(NT2):
                hA, hB = 2 * t2, 2 * t2 + 1
                pvA = ps_pv.tile([P, 4, VW], f32, tag="pvA", name="pvA")
                pvB = ps_pv.tile([P, 4, VW], f32, tag="pvB", name="pvB")
                heads = ((hA, 0, pvA), (hB, 64, pvB))
                started = {0: False, 1: False}
                exs = {}
                pending = []
                fq = list(fillers[t2])
                # spread fillers across the pair's units
                per_u = [[] for _ in range(8)]
                for k, f in enumerate(fq):
                    per_u[(k * 8) // len(fq) if len(fq) > 8 else k % 8].append(f)

                def emit_pv(u, last=False):
                    for hi, (h, _, pv) in enumerate(heads):
                        ex = exs.pop((u, hi))
                        for qc in range(4):
                            mm(
                                pv[:, qc, :],
                                ex[:, :, qc * P : (qc + 1) * P],
                                v_sb[:, 2 * u : 2 * u + 2, h, :],
                                start=not started[hi],
                                stop=last and qc == 3,
                                dr=True,
                            )
                            started[hi] = True

                for u in range(8):
                    for f in per_u[u]:
                        f()
                    for hi, (h, roff, pv) in enumerate(heads):
                        sc = ps_sc.tile([P, 2, SQ], f32, tag="sc")
                        for j in range(2):
                            c = 2 * u + j
                            mm(
                                sc[:, j, :],
                                kt[roff : roff + HD, t2, c * P : (c + 1) * P],
                                qt[roff : roff + HD, t2, :],
                                start=True, stop=True,
                                tile_position=(roff, 0),
                            )
                        ex = ex_pool.tile([P, 2, SQ], f8, tag="ex", name="ex")
                        if pick(1.313, 1.456) == 0:
                            nc.scalar.activation(ex[:], sc[:], EXP, scale=ALPHA)
                        else:
                            nc.vector.tensor_scalar(
                                out=ex[:].bitcast(i8), in0=sc[:],
                                scalar1=EXP_A, scalar2=EXP_B,
                                op0=MULT, op1=ADD,
                            )
                        exs[(u, hi)] = ex
                    pending.append(u)
                    if len(pending) > 2:
                        emit_pv(pending.pop(0))
                for k, u in enumerate(pending):
                    emit_pv(u, last=(k == len(pending) - 1))
                for h, _, pv in heads:
                    rcp = rcp_pool.tile([P, 4, 1], f32, tag="rcp", name="rcp")
                    nc.vector.reciprocal(rcp[:], pv[:, :, HD : HD + 1])
                    nc.vector.tensor_mul(
                        ctxq[:, :, h * HD : (h + 1) * HD],
                        pv[:, :, 0:HD],
                        rcp[:].to_broadcast((P, 4, HD)),
                    )

            # ---------- ctx reorientation + O-proj + LayerNorm ----------
            with (
                tc.tile_pool(name="lnconst", bufs=1) as lnc_pool,
                tc.tile_pool(name="xbuf", bufs=2) as xb_pool,
                tc.tile_pool(name="stats", bufs=4) as st_pool,
            ):
                g_sb = lnc_pool.tile([P, D], f32)
                nc.sync.dma_start(g_sb[:], gamma_bc[:])
                be_sb = lnc_pool.tile([P, D], f32)
                nc.sync.dma_start(be_sb[:], beta_bc[:])
                eps_sb = lnc_pool.tile([P, 1], f32)
                nc.gpsimd.memset(eps_sb[:], EPS)

                for qp in range(4):
                    for t2 in range(NT2):
                        pt = pw_pool.tile([P, P, 2], f8, tag="pw", name="pt")
                        nc.tensor.transpose(
                            pt[:, :, 0], ctxq[:, qp, t2 * P : (t2 + 1) * P], id_sb[:]
                        )
                        evac(ctxn[:, t2, qp * P : (qp + 1) * P], pt[:, :, 0])

                    po = ps_sc.tile([P, 2, SQ], f32, tag="sc", name="po")
                    for nd in range(2):
                        for i in range(4):
                            mm(
                                po[:, nd, :],
                                ctxn[:, 2 * i : 2 * i + 2, qp * P : (qp + 1) * P],
                                wo_sb[:, 2 * i : 2 * i + 2, nd * SQ : (nd + 1) * SQ],
                                start=(i == 0), stop=(i == 3), dr=True,
                            )
                    xbuf = xb_pool.tile([P, D], f32)
                    nc.vector.scalar_tensor_tensor(
                        out=xbuf[:],
                        in0=po[:].rearrange("p a d -> p (a d)"),
                        scalar=OSCALE,
                        in1=xq_sb[:, qp, :],
                        op0=MULT, op1=ADD,
                    )
                    stats = st_pool.tile([P, 2, 6], f32)
                    xbuf_v = xbuf[:].rearrange("p (a d) -> p a d", a=2)
                    for a in range(2):
                        nc.vector.bn_stats(stats[:, a, :], xbuf_v[:, a, :])
                    mv = st_pool.tile([P, 2], f32)
                    nc.vector.bn_aggr(mv[:], stats[:])
                    rstd = st_pool.tile([P, 1], f32)
                    nc.scalar.activation(
                        rstd[:],
                        mv[:, 1:2],
                        mybir.ActivationFunctionType.Sqrt,
                        bias=eps_sb[:],
                    )
                    nc.vector.reciprocal(rstd[:], rstd[:])
                    nc.vector.tensor_scalar(
                        out=xbuf[:],
                        in0=xbuf[:],
                        scalar1=mv[:, 0:1],
                        scalar2=rstd[:],
                        op0=mybir.AluOpType.subtract,
                        op1=MULT,
                    )
                    nc.gpsimd.tensor_mul(xbuf[:], xbuf[:], g_sb[:])
                    nc.gpsimd.tensor_add(xbuf[:], xbuf[:], be_sb[:])
                    nc.sync.dma_start(out_r[qp], xbuf[:])

    nc.finalize()
    return nc


def _shard_inputs(inputs):
    import ml_dtypes

    f8 = ml_dtypes.float8_e4m3
    bf = ml_dtypes.bfloat16

    x = np.ascontiguousarray(np.asarray(inputs["hidden_states"], dtype=np.float32))
    mask = np.asarray(inputs["attention_mask"], dtype=np.float32).reshape(B, S)

    def w8(name):
        w = np.asarray(inputs[name], dtype=np.float32) * WSCALE
        return np.ascontiguousarray(np.clip(w, -F8MAX, F8MAX).astype(f8))

    Wq8, Wk8, Wv8, Wo8 = w8("Wq"), w8("Wk"), w8("Wv"), w8("Wo")
    bq = np.asarray(inputs["bq"], dtype=np.float32)
    bk = np.asarray(inputs["bk"], dtype=np.float32)
    bv = np.asarray(inputs["bv"], dtype=np.float32)
    bo = np.asarray(inputs["bo"], dtype=np.float32)
    bq_c = np.ascontiguousarray((WSCALE * bq).reshape(DT, P).T)
    bk_c = np.ascontiguousarray((WSCALE * bk).reshape(DT, P).T)
    bv_r = np.ascontiguousarray((WSCALE * bv).reshape(1, D).astype(bf))
    gamma = np.asarray(inputs["ln_gamma"], dtype=np.float32)
    beta = np.asarray(inputs["ln_beta"], dtype=np.float32)
    gamma_bc = np.ascontiguousarray(np.broadcast_to(gamma, (P, D)))
    beta_bc = np.ascontiguousarray(np.broadcast_to(beta, (P, D)))
    ident = np.ascontiguousarray(np.eye(P, dtype=np.float32).astype(f8))

    em = np.exp(mask)

    in_maps = []
    for c in range(NCORES):
        b, q = c // 4, (c % 4) * SQ
        xb = np.roll(x[b], -q, axis=0)
        emb = np.roll(em[b], -q)
        xT8 = np.ascontiguousarray(np.clip(xb.T, -F8MAX, F8MAX).astype(f8))
        em_t = np.ascontiguousarray(emb.reshape(KS, P).T)
        in_maps.append(
            {
                "xT": xT8,
                "xq": np.ascontiguousarray(x[b, q : q + SQ, :] + bo),
                "Wq": Wq8, "Wk": Wk8, "Wv": Wv8, "Wo": Wo8,
                "bq_c": bq_c, "bk_c": bk_c, "bv_r": bv_r,
                "em_t": em_t,
                "ident": ident,
                "gamma_bc": gamma_bc, "beta_bc": beta_bc,
            }
        )
    return in_maps


def run(inputs, trace=False, **kw):
    _ensure_paths()
    from concourse.bass_utils import run_bass_kernel_spmd

    if "nc" not in _CACHE:
        _CACHE["nc"] = build_nc()
    nc = _CACHE["nc"]
    in_maps = _shard_inputs(inputs)
    res = run_bass_kernel_spmd(
        nc, in_maps, core_ids=list(range(NCORES)), trace=trace, **kw
    )
    parts = [res.results[c]["out"] for c in range(NCORES)]
    full = np.empty((B, S, D), dtype=np.float32)
    for c in range(NCORES):
        b, q = c // 4, (c % 4) * SQ
        full[b, q : q + SQ] = parts[c]
    return full, res


def kernel(**inputs):
    out, _ = run(inputs)
    return out
